# revision 1
# baseline (speedup 1.0000x reference)
"""Trainium2 Bass kernel for nn_AutoCorrelation_spa_tem.

Shards batch B=32 across 8 NeuronCores (4 batches/core, pure data parallel).

Algorithm (collapsed form of the reference):
  G_b   = keys[b](L,HE) @ queries[b](L,HE)^T            (192x192)
  D_raw[b,tau] = sum_s G_b[s,(s+tau)%L]                 (diag sums via shear)
  gsum  = AllReduce_b(D_raw)  -> top-5 mask via max8
  c_b   = mask * softmax(D_raw[b]/HE over selected)
  W_b   = keys[b].reshape(HE,L)^T @ values_proper(HE,L) (192x192)
  M_b   = sum_d c_b[d] * Shift_d(W_b)   [2D circular diagonal shift]
        = unshear(HankelC^T @ shear(W_b))   (all positive-stride DMAs)
  out[b] = (Qtilde_b @ M_b)^T  computed as Mrev^T @ qr  (qr host-row-reversed)
"""

import numpy as np

B, L, H, E = 32, 192, 8, 64
HE = H * E
N_CORES = 8
PER = B // N_CORES

_compiled = {}


def _build():
    import concourse.bacc as bacc
    import concourse.mybir as mybir
    from concourse.bass_types import AP
    from concourse.tile import TileContext

    dt = mybir.dt.float32
    dtr = mybir.dt.float32r
    nc = bacc.Bacc("TRN2", target_bir_lowering=False, debug=False,
                   num_devices=N_CORES, num_swdge_queues=4)

    kt = nc.dram_tensor("kt", [PER, HE, L], dt, kind="ExternalInput")
    qt = nc.dram_tensor("qt", [PER, HE, L], dt, kind="ExternalInput")
    kf = nc.dram_tensor("kf", [PER, HE, L], dt, kind="ExternalInput")
    vt = nc.dram_tensor("vt", [PER, HE, L], dt, kind="ExternalInput")
    qr = nc.dram_tensor("qr", [PER, L, HE], dt, kind="ExternalInput")
    onesin = nc.dram_tensor("ones_in", [128, 1], dt, kind="ExternalInput")
    out = nc.dram_tensor("out", [PER, L, HE], dt, kind="ExternalOutput")

    g3a = nc.dram_tensor("g3a", [PER * L * 576], dt)
    w3 = [nc.dram_tensor(f"w3_{b}", [L * 576], dt) for b in range(PER)]
    m3a = nc.dram_tensor("m3a", [PER * L * 576], dt)
    c3a = nc.dram_tensor("c3a", [PER * 576], dt)
    arin = nc.dram_tensor("arin", [1, L], dt)
    arout = nc.dram_tensor("arout", [1, L], dt, addr_space="Shared")

    PCH = [(0, 128), (128, 64)]
    Exp = mybir.ActivationFunctionType.Exp
    Alu = mybir.AluOpType
    Ax = mybir.AxisListType

    def load_4chunks(eng, tile_, src, b):
        # src[b] is (HE, L) contiguous; tile_ is (128, 4*L): chunk i at cols i*L
        return eng.dma_start(
            out=tile_[:, :].rearrange("p (i l) -> p i l", i=4),
            in_=AP(tensor=src, offset=b * HE * L,
                   ap=[[L, 128], [128 * L, 4], [1, L]]).bitcast(dtr))

    with TileContext(nc) as tc:
        with tc.tile_pool(name="sb", bufs=1) as sb, \
             tc.tile_pool(name="ps", bufs=1, space="PSUM") as ps:

            # ---------- input loads (all up front; sync HWDGE) ----------
            kt_t, qt_t, kf_t, vt_t, qr_t = {}, {}, {}, {}, {}
            for b in range(PER):
                kt_t[b] = sb.tile([128, 4 * L], dtr, tag=f"kt{b}", name=f"kt{b}")
                qt_t[b] = sb.tile([128, 4 * L], dtr, tag=f"qt{b}", name=f"qt{b}")
                load_4chunks(nc.sync, kt_t[b], kt, b)
                load_4chunks(nc.gpsimd, qt_t[b], qt, b)

            # ---------- G_b -> per-b shear-write / shear-read (pipelined) ----------
            gsh = {}
            BSTR = L * 576
            for b in range(PER):
                for (m0, mn) in PCH:
                    gp = ps.tile([mn, L], dt, tag="mm", bufs=4)
                    for i in range(4):
                        nc.tensor.matmul(gp[:, :],
                                         kt_t[b][:, i*L + m0 : i*L + m0 + mn],
                                         qt_t[b][:, i*L:(i+1)*L],
                                         start=(i == 0), stop=(i == 3))
                    gs = sb.tile([mn, 2 * L], dt, tag=f"gs{m0}", bufs=2, name=f"gs{b}_{m0}")
                    nc.vector.tensor_copy(gs[:, :].rearrange("p (r l) -> p r l", r=2),
                                          gp[:, :].unsqueeze(1).broadcast_to((mn, 2, L)))
                    nc.sync.dma_start(
                        out=AP(tensor=g3a, offset=b * BSTR + 192 + m0 * 575,
                               ap=[[575, mn], [1, 2 * L]]),
                        in_=gs[:, :])
                for (m0, mn) in PCH:
                    t = sb.tile([mn, 4 * L], dtr, tag=f"gsh{m0}", name=f"gsh{m0}_{b}")
                    nc.gpsimd.dma_start(
                        out=t[:, b*L:(b+1)*L],
                        in_=AP(tensor=g3a, offset=b * BSTR + 192 + m0 * 576,
                               ap=[[576, mn], [1, L]]).bitcast(dtr))
                    gsh[(b, m0)] = t

            # ---------- D_raw ----------
            ones_t = sb.tile([128, 1], dtr, tag="ones")
            nc.gpsimd.dma_start(out=ones_t[:, :], in_=onesin[:, :].bitcast(dtr))
            d_sb = sb.tile([1, 4 * L], dt, tag="d")
            for b in range(PER):
                dpb = ps.tile([1, L], dt, tag="dp", bufs=2)
                for i, (m0, mn) in enumerate(PCH):
                    nc.tensor.matmul(dpb[:, :], ones_t[:mn, :],
                                     gsh[(b, m0)][:, b*L:(b+1)*L],
                                     start=(i == 0), stop=(i == 1))
                nc.vector.tensor_copy(d_sb[:, b*L:(b+1)*L], dpb[:, :])

            part = sb.tile([1, L], dt, tag="part")
            nc.vector.tensor_add(part[:, :], d_sb[:, 0:L], d_sb[:, L:2*L])
            part2 = sb.tile([1, L], dt, tag="part2")
            nc.vector.tensor_add(part2[:, :], d_sb[:, 2*L:3*L], d_sb[:, 3*L:4*L])
            nc.vector.tensor_add(part[:, :], part[:, :], part2[:, :])
            arin_inst = nc.gpsimd.dma_start(out=arin[:, :], in_=part[:, :])
            nc.gpsimd.collective_compute(
                "AllReduce", Alu.add,
                replica_groups=[list(range(N_CORES))],
                ins=[arin[:, :]], outs=[arout[:, :]])

            # ---------- pre-CC: d4 + exp (safe without max-subtraction:
            # |D_raw/HE| <~ 3 for this data distribution) ----------
            d4 = sb.tile([PER, L], dt, tag="d4")
            nc.gpsimd.dma_start(
                out=d4[:, :],
                in_=d_sb[:, :].rearrange("p (b l) -> p b l", b=PER))
            e4 = sb.tile([PER, L], dt, tag="e4")
            nc.scalar.activation(e4[:, :], d4[:, :], Exp, bias=0.0, scale=1.0 / HE)

            # ---------- post-CC: mask + normalize -> c (PER x L) ----------
            gsum4 = sb.tile([PER, L], dt, tag="gsum4")
            nc.gpsimd.dma_start(out=gsum4[:, :],
                                in_=AP(tensor=arout, offset=0, ap=[[0, PER], [1, L]]))
            mx = sb.tile([PER, 8], dt, tag="mx")
            nc.vector.max(out=mx[:, :], in_=gsum4[:, :])
            me = sb.tile([PER, L], dt, tag="me")
            nc.vector.tensor_scalar(out=me[:, :], in0=gsum4[:, :],
                                    scalar1=mx[:, 4:5], scalar2=None, op0=Alu.is_ge)
            nc.vector.tensor_tensor(out=me[:, :], in0=me[:, :], in1=e4[:, :], op=Alu.mult)
            z = sb.tile([PER, 1], dt, tag="z")
            nc.vector.tensor_reduce(out=z[:, :], in_=me[:, :], axis=Ax.X, op=Alu.add)
            zr = sb.tile([PER, 1], dt, tag="zr")
            nc.vector.reciprocal(zr[:, :], z[:, :])
            c4 = sb.tile([PER, L], dt, tag="c4")
            nc.vector.tensor_scalar(out=c4[:, :], in0=me[:, :], scalar1=zr[:, :],
                                    scalar2=None, op0=Alu.mult)

            # ---------- W-phase loads (delayed behind CC input so the
            # pre-collective window stays clear for the G/D critical path) ----------
            from concourse.tile import add_dep_helper
            for b in range(PER):
                kf_t[b] = sb.tile([128, 4 * L], dtr, tag=f"kf{b}", name=f"kf{b}")
                vt_t[b] = sb.tile([128, 4 * L], dtr, tag=f"vt{b}", name=f"vt{b}")
                i1 = load_4chunks(nc.sync, kf_t[b], kf, b)
                i2 = load_4chunks(nc.sync, vt_t[b], vt, b)
                qr_t[b] = sb.tile([128, 2 * HE], dtr, tag=f"qr{b}", name=f"qr{b}")
                i3 = nc.sync.dma_start(out=qr_t[b][:, 0:HE], in_=qr[b, 0:128, :].bitcast(dtr))
                i4 = nc.sync.dma_start(out=qr_t[b][:64, HE:2*HE], in_=qr[b, 128:192, :].bitcast(dtr))
                for ii in (i1, i2, i3, i4):
                    add_dep_helper(ii.ins, arin_inst.ins, sync=True,
                                   reason="delay W loads past CC input")

            # ---------- W_b -> Wsh (overlaps collective flight) ----------
            wsh = {}
            for b in range(PER):
                for (m0, mn) in PCH:
                    wp = ps.tile([mn, L], dt, tag="mm", bufs=4)
                    for i in range(4):
                        nc.tensor.matmul(wp[:, :],
                                         kf_t[b][:, i*L + m0 : i*L + m0 + mn],
                                         vt_t[b][:, i*L:(i+1)*L],
                                         start=(i == 0), stop=(i == 3))
                    ws = sb.tile([mn, 2 * L], dt, tag=f"ws{m0}", bufs=2, name=f"ws{b}_{m0}")
                    nc.vector.tensor_copy(ws[:, :].rearrange("p (r l) -> p r l", r=2),
                                          wp[:, :].unsqueeze(1).broadcast_to((mn, 2, L)))
                    nc.sync.dma_start(
                        out=AP(tensor=w3[b], offset=192 + m0 * 575,
                               ap=[[575, mn], [1, 2 * L]]),
                        in_=ws[:, :])
                for (m0, mn) in PCH:
                    t = sb.tile([mn, L], dtr, tag=f"wsh{b}_{m0}")
                    nc.sync.dma_start(
                        out=t[:, :],
                        in_=AP(tensor=w3[b], offset=192 + m0 * 576,
                               ap=[[576, mn], [1, L]]).bitcast(dtr))
                    wsh[(b, m0)] = t

            # ---------- c3 (one write), H1 (two reads) ----------
            nc.gpsimd.dma_start(
                out=AP(tensor=c3a, offset=0, ap=[[576, PER], [192, 3], [1, L]]),
                in_=c4[:, :].unsqueeze(1).broadcast_to((PER, 3, L)))
            h1 = {}
            for (m0, mn) in PCH:
                t = sb.tile([mn, PER * L], dtr, tag=f"h1_{m0}", name=f"h1_{m0}")
                nc.gpsimd.dma_start(
                    out=t[:, :].rearrange("p (b l) -> p b l", b=PER),
                    in_=AP(tensor=c3a, offset=1 + m0,
                           ap=[[1, mn], [576, PER], [1, L]]).bitcast(dtr))
                h1[m0] = t

            # ---------- T1, Mrev, final per b ----------
            MSTR = L * 576
            for b in range(PER):
                for (m0, mn) in PCH:
                    tp = ps.tile([mn, L], dt, tag="mm", bufs=4)
                    for i, (u0, un) in enumerate(PCH):
                        nc.tensor.matmul(tp[:, :], h1[u0][:, b*L + m0 : b*L + m0 + mn],
                                         wsh[(b, u0)][:, :],
                                         start=(i == 0), stop=(i == 1))
                    ts_ = sb.tile([mn, 2 * L], dt, tag=f"ts{m0}", bufs=2, name=f"ts{b}_{m0}")
                    nc.vector.tensor_copy(ts_[:, :].rearrange("p (r l) -> p r l", r=2),
                                          tp[:, :].unsqueeze(1).broadcast_to((mn, 2, L)))
                    nc.sync.dma_start(
                        out=AP(tensor=m3a, offset=b * MSTR + 191 + m0 * 575,
                               ap=[[575, mn], [1, 2 * L]]),
                        in_=ts_[:, :])
                mrev = {}
                for (m0, mn) in PCH:
                    t = sb.tile([mn, L], dtr, tag=f"mrev{m0}", bufs=2, name=f"mrev{b}_{m0}")
                    nc.gpsimd.dma_start(
                        out=t[:, :],
                        in_=AP(tensor=m3a, offset=b * MSTR + 192 + m0 * 576,
                               ap=[[576, mn], [1, L]]).bitcast(dtr))
                    mrev[m0] = t

                for (l0, ln) in PCH:
                    op_ = ps.tile([ln, HE], dt, tag="op", bufs=2)
                    for i, (i0, in_n) in enumerate(PCH):
                        nc.tensor.matmul(op_[:, :], mrev[i0][:, l0:l0+ln],
                                         qr_t[b][:in_n, i*HE:(i+1)*HE],
                                         start=(i == 0), stop=(i == 1))
                    os_ = sb.tile([ln, HE], dt, tag=f"os{l0}", bufs=2, name=f"os{b}_{l0}")
                    nc.vector.tensor_copy(os_[:, :], op_[:, :])
                    nc.sync.dma_start(out=out[b, l0:l0+ln, :], in_=os_[:, :])

    nc.finalize()
    return nc


def _get_nc():
    if "nc" not in _compiled:
        _compiled["nc"] = _build()
    return _compiled["nc"]


def kernel(queries, keys, values, adj, attn_mask):
    from concourse.bass_utils import run_bass_kernel_spmd

    queries = np.ascontiguousarray(np.asarray(queries, dtype=np.float32))
    keys = np.ascontiguousarray(np.asarray(keys, dtype=np.float32))
    values = np.ascontiguousarray(np.asarray(values, dtype=np.float32))

    nc = _get_nc()
    in_maps = []
    for c in range(N_CORES):
        sl = slice(c * PER, (c + 1) * PER)
        q, k, v = queries[sl], keys[sl], values[sl]
        in_maps.append({
            "kt": np.ascontiguousarray(k.reshape(PER, L, HE).transpose(0, 2, 1)),
            "qt": np.ascontiguousarray(q.reshape(PER, L, HE).transpose(0, 2, 1)),
            "kf": np.ascontiguousarray(k.reshape(PER, HE, L)),
            "vt": np.ascontiguousarray(v.reshape(PER, L, HE).transpose(0, 2, 1)),
            "qr": np.ascontiguousarray(
                q.reshape(PER, HE, L).transpose(0, 2, 1)[:, ::-1, :]),
            "ones_in": np.ones((128, 1), dtype=np.float32),
        })

    res = run_bass_kernel_spmd(nc, in_maps, list(range(N_CORES)),
                               **_compiled.get("run_kwargs", {}))
    _compiled["last_result"] = res
    outs = [res.results[c]["out"].reshape(PER, L, H, E) for c in range(N_CORES)]
    return np.concatenate(outs, axis=0)



# revision 3
# speedup vs baseline: 1.1992x; 1.1992x over previous
"""Trainium2 Bass kernel for nn_AutoCorrelation_spa_tem.

Shards batch B=32 across 8 NeuronCores (4 batches/core, pure data parallel).

Algorithm (collapsed form of the reference):
  G_b   = keys[b](L,HE) @ queries[b](L,HE)^T            (192x192)
  D_raw[b,tau] = sum_s G_b[s,(s+tau)%L]                 (diag sums via shear)
  gsum  = AllReduce_b(D_raw)  -> top-5 mask via max8
  c_b   = mask * softmax(D_raw[b]/HE over selected)
  W_b   = keys[b].reshape(HE,L)^T @ values_proper(HE,L) (192x192)
  M_b   = sum_d c_b[d] * Shift2D_d(W_b)
        = unshear(HankelC^T @ shear(W_b))
  out[b] = (Qtilde_b @ M_b)^T  computed as Mrev^T @ qr  (qr host-row-reversed)

v2: bf16 operands throughout (fp32 PSUM accum), host-packed SBUF-layout
inputs for large HWDGE descriptors, contiguous doubled-row shear buffers
(write stride 384 == row width -> fully contiguous writes; diagonal read
stride 385), collective triggered as early as possible on an otherwise
empty gpsimd queue, W phase loads/compute overlapping the collective
flight, bf16 output.
"""

import numpy as np
import ml_dtypes

B, L, H, E = 32, 192, 8, 64
HE = H * E
N_CORES = 8
PER = B // N_CORES
BF = ml_dtypes.bfloat16

_compiled = {}


def _build():
    import concourse.bacc as bacc
    import concourse.mybir as mybir
    from concourse.bass_types import AP
    from concourse.tile import TileContext, add_dep_helper

    dt = mybir.dt.float32
    dtb = mybir.dt.bfloat16
    nc = bacc.Bacc("TRN2", target_bir_lowering=False, debug=False,
                   num_devices=N_CORES, num_swdge_queues=4)

    # host-packed inputs: per batch, (128, 4*L) chunk layout; qr (128, 2*HE)
    kt = nc.dram_tensor("kt", [PER, 128, 4 * L], dtb, kind="ExternalInput")
    qt = nc.dram_tensor("qt", [PER, 128, 4 * L], dtb, kind="ExternalInput")
    kf = nc.dram_tensor("kf", [PER, 128, 4 * L], dtb, kind="ExternalInput")
    vt = nc.dram_tensor("vt", [PER, 128, 4 * L], dtb, kind="ExternalInput")
    qr = nc.dram_tensor("qr", [PER, 128, 2 * HE], dtb, kind="ExternalInput")
    onesin = nc.dram_tensor("ones_in", [128, 1], dtb, kind="ExternalInput")
    out = nc.dram_tensor("out", [PER, L, HE], dtb, kind="ExternalOutput")

    # doubled-row shear scratch: row m of batch b at b*L*384 + m*384,
    # content [row, row] (384 wide) -> contiguous writes, diag reads @385
    BSTR = L * 384
    gsc = nc.dram_tensor("gsc", [PER * BSTR], dtb)
    wsc = nc.dram_tensor("wsc", [PER * BSTR], dtb)
    msc = nc.dram_tensor("msc", [PER * BSTR], dtb)
    csc = nc.dram_tensor("csc", [PER * 576], dtb)
    arin = nc.dram_tensor("arin", [1, L], dt)
    arout = nc.dram_tensor("arout", [1, L], dt, addr_space="Shared")

    PCH = [(0, 128), (128, 64)]
    Exp = mybir.ActivationFunctionType.Exp
    Alu = mybir.AluOpType
    Ax = mybir.AxisListType

    with TileContext(nc) as tc:
        with tc.tile_pool(name="sb", bufs=1) as sb, \
             tc.tile_pool(name="ps", bufs=1, space="PSUM") as ps:

            # ---------- G-phase input loads (per batch, 2 HWDGE queues) ----
            kt_t, qt_t, kf_t, vt_t, qr_t = {}, {}, {}, {}, {}
            for b in range(PER):
                kt_t[b] = sb.tile([128, 4 * L], dtb, tag=f"kt{b}", name=f"kt{b}")
                qt_t[b] = sb.tile([128, 4 * L], dtb, tag=f"qt{b}", name=f"qt{b}")
                nc.sync.dma_start(out=kt_t[b][:, :], in_=kt[b, :, :])
                nc.scalar.dma_start(out=qt_t[b][:, :], in_=qt[b, :, :])
            ones_t = sb.tile([128, 1], dtb, tag="ones")
            nc.sync.dma_start(out=ones_t[:, :], in_=onesin[:, :])

            # ---------- G_b -> contiguous doubled-row write, diag read -----
            gsh = {}
            for b in range(PER):
                for (m0, mn) in PCH:
                    gp = ps.tile([mn, L], dt, tag="mm", bufs=4)
                    for i in range(4):
                        nc.tensor.matmul(gp[:, :],
                                         kt_t[b][:, i*L + m0 : i*L + m0 + mn],
                                         qt_t[b][:, i*L:(i+1)*L],
                                         start=(i == 0), stop=(i == 3))
                    gs = sb.tile([mn, 2 * L], dtb, tag=f"gs{m0}", bufs=2,
                                 name=f"gs{b}_{m0}")
                    nc.vector.tensor_copy(
                        gs[:, :].rearrange("p (r l) -> p r l", r=2),
                        gp[:, :].unsqueeze(1).broadcast_to((mn, 2, L)))
                    nc.sync.dma_start(
                        out=AP(tensor=gsc, offset=b * BSTR + m0 * 384,
                               ap=[[384, mn], [1, 2 * L]]),
                        in_=gs[:, :])
                for (m0, mn) in PCH:
                    t = sb.tile([mn, L], dtb, tag=f"gsh{b}_{m0}")
                    nc.sync.dma_start(
                        out=t[:, :],
                        in_=AP(tensor=gsc, offset=b * BSTR + m0 * 385,
                               ap=[[385, mn], [1, L]]))
                    gsh[(b, m0)] = t

            # ---------- D_raw ----------
            d_sb = sb.tile([1, PER * L], dt, tag="d")
            for b in range(PER):
                dpb = ps.tile([1, L], dt, tag="dp", bufs=2)
                for i, (m0, mn) in enumerate(PCH):
                    nc.tensor.matmul(dpb[:, :], ones_t[:mn, 0:1],
                                     gsh[(b, m0)][:, :],
                                     start=(i == 0), stop=(i == 1))
                nc.vector.tensor_copy(d_sb[:, b*L:(b+1)*L], dpb[:, :])

            part = sb.tile([1, L], dt, tag="part")
            nc.vector.tensor_add(part[:, :], d_sb[:, 0:L], d_sb[:, L:2*L])
            part2 = sb.tile([1, L], dt, tag="part2")
            nc.vector.tensor_add(part2[:, :], d_sb[:, 2*L:3*L], d_sb[:, 3*L:4*L])
            nc.vector.tensor_add(part[:, :], part[:, :], part2[:, :])
            arin_inst = nc.gpsimd.dma_start(out=arin[:, :], in_=part[:, :])
            nc.gpsimd.collective_compute(
                "AllReduce", Alu.add,
                replica_groups=[list(range(N_CORES))],
                ins=[arin[:, :]], outs=[arout[:, :]])

            # ---------- during-CC: d4 scatter + exp ----------
            d4 = sb.tile([PER, L], dt, tag="d4")
            nc.gpsimd.dma_start(
                out=d4[:, :],
                in_=d_sb[:, :].rearrange("p (b l) -> p b l", b=PER))
            e4 = sb.tile([PER, L], dt, tag="e4")
            nc.scalar.activation(e4[:, :], d4[:, :], Exp, bias=0.0, scale=1.0 / HE)

            # ---------- W-phase loads (delayed behind CC input) ----------
            for b in range(PER):
                kf_t[b] = sb.tile([128, 4 * L], dtb, tag=f"kf{b}", name=f"kf{b}")
                vt_t[b] = sb.tile([128, 4 * L], dtb, tag=f"vt{b}", name=f"vt{b}")
                qr_t[b] = sb.tile([128, 2 * HE], dtb, tag=f"qr{b}", name=f"qr{b}")
                i1 = nc.sync.dma_start(out=kf_t[b][:, :], in_=kf[b, :, :])
                i2 = nc.scalar.dma_start(out=vt_t[b][:, :], in_=vt[b, :, :])
                i3 = nc.scalar.dma_start(out=qr_t[b][:, :], in_=qr[b, :, :])
                for ii in (i1, i2, i3):
                    add_dep_helper(ii.ins, arin_inst.ins, sync=True,
                                   reason="delay W loads past CC input")

            # ---------- W_b -> doubled-row write, diag read (in CC flight) --
            wsh = {}
            for b in range(PER):
                for (m0, mn) in PCH:
                    wp = ps.tile([mn, L], dt, tag="mm", bufs=4)
                    for i in range(4):
                        nc.tensor.matmul(wp[:, :],
                                         kf_t[b][:, i*L + m0 : i*L + m0 + mn],
                                         vt_t[b][:, i*L:(i+1)*L],
                                         start=(i == 0), stop=(i == 3))
                    ws = sb.tile([mn, 2 * L], dtb, tag=f"ws{m0}", bufs=2,
                                 name=f"ws{b}_{m0}")
                    nc.vector.tensor_copy(
                        ws[:, :].rearrange("p (r l) -> p r l", r=2),
                        wp[:, :].unsqueeze(1).broadcast_to((mn, 2, L)))
                    nc.sync.dma_start(
                        out=AP(tensor=wsc, offset=b * BSTR + m0 * 384,
                               ap=[[384, mn], [1, 2 * L]]),
                        in_=ws[:, :])
                for (m0, mn) in PCH:
                    t = sb.tile([mn, L], dtb, tag=f"wsh{b}_{m0}")
                    nc.sync.dma_start(
                        out=t[:, :],
                        in_=AP(tensor=wsc, offset=b * BSTR + m0 * 385,
                               ap=[[385, mn], [1, L]]))
                    wsh[(b, m0)] = t

            # ---------- post-CC: mask + normalize -> c (PER x L) ----------
            gsum4 = sb.tile([PER, L], dt, tag="gsum4")
            nc.gpsimd.dma_start(out=gsum4[:, :],
                                in_=AP(tensor=arout, offset=0,
                                       ap=[[0, PER], [1, L]]))
            mx = sb.tile([PER, 8], dt, tag="mx")
            nc.vector.max(out=mx[:, :], in_=gsum4[:, :])
            me = sb.tile([PER, L], dt, tag="me")
            nc.vector.tensor_scalar(out=me[:, :], in0=gsum4[:, :],
                                    scalar1=mx[:, 4:5], scalar2=None,
                                    op0=Alu.is_ge)
            nc.vector.tensor_tensor(out=me[:, :], in0=me[:, :], in1=e4[:, :],
                                    op=Alu.mult)
            z = sb.tile([PER, 1], dt, tag="z")
            nc.vector.tensor_reduce(out=z[:, :], in_=me[:, :], axis=Ax.X,
                                    op=Alu.add)
            zr = sb.tile([PER, 1], dt, tag="zr")
            nc.vector.reciprocal(zr[:, :], z[:, :])
            c4 = sb.tile([PER, L], dt, tag="c4")
            nc.vector.tensor_scalar(out=c4[:, :], in0=me[:, :],
                                    scalar1=zr[:, :], scalar2=None,
                                    op0=Alu.mult)
            # tripled bf16 copy in SBUF -> one contiguous DRAM write
            c4t = sb.tile([PER, 3 * L], dtb, tag="c4t")
            nc.vector.tensor_copy(
                c4t[:, :].rearrange("p (r l) -> p r l", r=3),
                c4[:, :].unsqueeze(1).broadcast_to((PER, 3, L)))
            nc.sync.dma_start(
                out=AP(tensor=csc, offset=0, ap=[[576, PER], [1, 3 * L]]),
                in_=c4t[:, :])
            h1 = {}
            for (m0, mn) in PCH:
                t = sb.tile([mn, PER * L], dtb, tag=f"h1_{m0}", name=f"h1_{m0}")
                nc.sync.dma_start(
                    out=t[:, :].rearrange("p (b l) -> p b l", b=PER),
                    in_=AP(tensor=csc, offset=1 + m0,
                           ap=[[1, mn], [576, PER], [1, L]]))
                h1[m0] = t

            # ---------- T1, mrev, final per b ----------
            for b in range(PER):
                for (m0, mn) in PCH:
                    tp = ps.tile([mn, L], dt, tag="mm", bufs=4)
                    for i, (u0, un) in enumerate(PCH):
                        nc.tensor.matmul(tp[:, :],
                                         h1[u0][:, b*L + m0 : b*L + m0 + mn],
                                         wsh[(b, u0)][:, :],
                                         start=(i == 0), stop=(i == 1))
                    ts_ = sb.tile([mn, 2 * L], dtb, tag=f"ts{m0}", bufs=2,
                                  name=f"ts{b}_{m0}")
                    nc.vector.tensor_copy(
                        ts_[:, :].rearrange("p (r l) -> p r l", r=2),
                        tp[:, :].unsqueeze(1).broadcast_to((mn, 2, L)))
                    nc.scalar.dma_start(
                        out=AP(tensor=msc, offset=b * BSTR + m0 * 384,
                               ap=[[384, mn], [1, 2 * L]]),
                        in_=ts_[:, :])
                mrev = {}
                for (m0, mn) in PCH:
                    t = sb.tile([mn, L], dtb, tag=f"mrev{m0}", bufs=2,
                                name=f"mrev{b}_{m0}")
                    nc.scalar.dma_start(
                        out=t[:, :],
                        in_=AP(tensor=msc, offset=b * BSTR + m0 * 385 + 1,
                               ap=[[385, mn], [1, L]]))
                    mrev[m0] = t

                for (l0, ln) in PCH:
                    op_ = ps.tile([ln, HE], dt, tag="op", bufs=2)
                    for i, (i0, in_n) in enumerate(PCH):
                        nc.tensor.matmul(op_[:, :], mrev[i0][:, l0:l0+ln],
                                         qr_t[b][:in_n, i*HE:(i+1)*HE],
                                         start=(i == 0), stop=(i == 1))
                    os_ = sb.tile([ln, HE], dtb, tag=f"os{l0}", bufs=2,
                                  name=f"os{b}_{l0}")
                    nc.vector.tensor_copy(os_[:, :], op_[:, :])
                    nc.sync.dma_start(out=out[b, l0:l0+ln, :], in_=os_[:, :])

    nc.finalize()
    return nc


def _get_nc():
    if "nc" not in _compiled:
        _compiled["nc"] = _build()
    return _compiled["nc"]


def _pack_chunks(mat):
    # (HE, L) -> (128, 4*L): column block i holds channels [i*128,(i+1)*128)
    return np.ascontiguousarray(
        mat.reshape(4, 128, L).transpose(1, 0, 2).reshape(128, 4 * L))


def kernel(queries, keys, values, adj, attn_mask):
    from concourse.bass_utils import run_bass_kernel_spmd

    queries = np.ascontiguousarray(np.asarray(queries, dtype=np.float32))
    keys = np.ascontiguousarray(np.asarray(keys, dtype=np.float32))
    values = np.ascontiguousarray(np.asarray(values, dtype=np.float32))

    nc = _get_nc()
    in_maps = []
    for c in range(N_CORES):
        sl = slice(c * PER, (c + 1) * PER)
        q = queries[sl].reshape(PER, L, HE)
        k = keys[sl].reshape(PER, L, HE)
        v = values[sl]
        kt = np.empty((PER, 128, 4 * L), BF)
        qt = np.empty((PER, 128, 4 * L), BF)
        kfp = np.empty((PER, 128, 4 * L), BF)
        vtp = np.empty((PER, 128, 4 * L), BF)
        qrp = np.zeros((PER, 128, 2 * HE), BF)
        for b in range(PER):
            kt[b] = _pack_chunks(k[b].T.astype(BF))
            qt[b] = _pack_chunks(q[b].T.astype(BF))
            kfp[b] = _pack_chunks(k[b].reshape(HE, L).astype(BF))
            vtp[b] = _pack_chunks(v[b].transpose(1, 2, 0)
                                  .reshape(HE, L).astype(BF))
            qsp = q[b].reshape(HE, L)
            qr_mat = qsp.T[::-1, :].astype(BF)   # (L, HE) reversed rows
            qrp[b, :, 0:HE] = qr_mat[0:128, :]
            qrp[b, 0:64, HE:2*HE] = qr_mat[128:192, :]
        in_maps.append({
            "kt": kt, "qt": qt, "kf": kfp, "vt": vtp, "qr": qrp,
            "ones_in": np.ones((128, 1), BF),
        })

    res = run_bass_kernel_spmd(nc, in_maps, list(range(N_CORES)),
                               **_compiled.get("run_kwargs", {}))
    _compiled["last_result"] = res
    outs = [np.asarray(res.results[c]["out"], dtype=np.float32)
            .reshape(PER, L, H, E) for c in range(N_CORES)]
    return np.concatenate(outs, axis=0)


# revision 9
# speedup vs baseline: 1.2147x; 1.0130x over previous
"""Trainium2 Bass kernel for nn_AutoCorrelation_spa_tem.

Shards batch B=32 across 8 NeuronCores (4 batches/core, pure data parallel).

Algorithm (collapsed form of the reference):
  G_b   = keys[b](L,HE) @ queries[b](L,HE)^T            (192x192)
  D_raw[b,tau] = sum_s G_b[s,(s+tau)%L]                 (diag sums via shear)
  gsum  = AllReduce_b(D_raw)  -> top-5 mask via max8
  c_b   = mask * softmax(D_raw[b]/HE over selected)
  W_b   = keys[b].reshape(HE,L)^T @ values_proper(HE,L) (192x192)
  M_b   = sum_d c_b[d] * Shift2D_d(W_b)
        = unshear(HankelC^T @ shear(W_b))
  out[b] = (Qtilde_b @ M_b)^T  computed as Mrev^T @ qr  (qr host-row-reversed)

v3: bf16 everywhere (fp32 PSUM), host-packed inputs, contiguous
doubled-row shear buffers (write stride 384 == row width, diagonal read
stride 385), few large HWDGE DMAs balanced across the SP/Act queues,
collective on an otherwise-empty gpsimd queue, W phase inside the CC
flight, PE p-state held up by dummy matmuls through every idle window,
tail ordered T1s-first so the msc/mrev roundtrips pipeline.
"""

import numpy as np
import ml_dtypes

B, L, H, E = 32, 192, 8, 64
HE = H * E
N_CORES = 8
PER = B // N_CORES
BF = ml_dtypes.bfloat16

_compiled = {}


def _build():
    import concourse.bacc as bacc
    import concourse.mybir as mybir
    from concourse.bass_types import AP
    from concourse.tile import TileContext, add_dep_helper

    dt = mybir.dt.float32
    dtb = mybir.dt.bfloat16
    nc = bacc.Bacc("TRN2", target_bir_lowering=False, debug=False,
                   num_devices=N_CORES, num_swdge_queues=4)

    kt = nc.dram_tensor("kt", [128, PER * 4 * L], dtb, kind="ExternalInput")
    qt = nc.dram_tensor("qt", [128, PER * 4 * L], dtb, kind="ExternalInput")
    kf = nc.dram_tensor("kf", [128, PER * 4 * L], dtb, kind="ExternalInput")
    vt = nc.dram_tensor("vt", [128, PER * 4 * L], dtb, kind="ExternalInput")
    qr = nc.dram_tensor("qr", [128, PER * 2 * HE], dtb, kind="ExternalInput")
    onesin = nc.dram_tensor("ones_in", [128, 512], dtb, kind="ExternalInput")
    out = nc.dram_tensor("out", [PER, L, HE], dtb, kind="ExternalOutput")

    BSTR = L * 384
    gsc = nc.dram_tensor("gsc", [PER * BSTR], dtb)
    wsc = nc.dram_tensor("wsc", [PER * BSTR], dtb)
    msc = nc.dram_tensor("msc", [PER * BSTR + 24832], dtb)
    csc = nc.dram_tensor("csc", [PER * 576], dtb)
    arin = nc.dram_tensor("arin", [1, L], dt)
    arout = nc.dram_tensor("arout", [1, L], dt, addr_space="Shared")

    PCH = [(0, 128), (128, 64)]
    Exp = mybir.ActivationFunctionType.Exp
    Alu = mybir.AluOpType
    Ax = mybir.AxisListType

    with TileContext(nc) as tc:
        with tc.tile_pool(name="sb", bufs=1) as sb, \
             tc.tile_pool(name="ps", bufs=1, space="PSUM") as ps:

            # ---------- inputs: one big HWDGE DMA per tensor ----------
            ones_t = sb.tile([128, 512], dtb, tag="ones")
            nc.sync.dma_start(out=ones_t[:, :], in_=onesin[:, :])
            kt_t = sb.tile([128, PER * 4 * L], dtb, tag="ktA", name="ktA")
            qt_t = sb.tile([128, PER * 4 * L], dtb, tag="qtA", name="qtA")
            nc.sync.dma_start(out=kt_t[:, :], in_=kt[:, :])
            nc.scalar.dma_start(out=qt_t[:, :], in_=qt[:, :])

            def slc(tile_, b, lo, hi):
                return tile_[:, b * 4 * L + lo : b * 4 * L + hi]

            # ---------- PE warm-up while loads are in flight ----------
            warm = ps.tile([1, 512], dt, tag="warm", bufs=1)

            def dummies(n, rows=512):
                for _ in range(n):
                    nc.tensor.matmul(warm[:, :rows], ones_t[:128, 0:1],
                                     ones_t[:, 0:rows], start=True, stop=True)

            dummies(10)

            # ---------- G_b -> doubled-row write (queue-split), diag read --
            for b in range(PER):
                for qi, (m0, mn) in enumerate(PCH):
                    gp = ps.tile([mn, L], dt, tag="mm", bufs=3)
                    for i in range(4):
                        nc.tensor.matmul(
                            gp[:, :],
                            slc(kt_t, b, i*L + m0, i*L + m0 + mn),
                            slc(qt_t, b, i*L, (i+1)*L),
                            start=(i == 0), stop=(i == 3))
                    gs = sb.tile([mn, 2 * L], dtb, tag=f"gs{m0}", bufs=2,
                                 name=f"gs{b}_{m0}")
                    nc.vector.tensor_copy(
                        gs[:, :].rearrange("p (r l) -> p r l", r=2),
                        gp[:, :].unsqueeze(1).broadcast_to((mn, 2, L)))
                    eng = nc.sync if qi == 0 else nc.scalar
                    eng.dma_start(
                        out=AP(tensor=gsc, offset=b * BSTR + m0 * 384,
                               ap=[[384, mn], [1, 2 * L]]),
                        in_=gs[:, :])
            gshA = {}
            for qi, (m0, mn) in enumerate(PCH):
                t = sb.tile([mn, PER * L], dtb, tag=f"gshA{m0}")
                eng = nc.sync if qi == 0 else nc.scalar
                eng.dma_start(
                    out=t[:, :].rearrange("p (b l) -> p b l", b=PER),
                    in_=AP(tensor=gsc, offset=m0 * 385,
                           ap=[[385, mn], [BSTR, PER], [1, L]]))
                gshA[m0] = t

            dummies(12)

            # ---------- D_raw: two (1,384) accumulations ----------
            d_sb = sb.tile([1, PER * L], dt, tag="d")
            for g in range(2):
                dp = ps.tile([1, 2 * L], dt, tag="dp", bufs=2)
                for i, (m0, mn) in enumerate(PCH):
                    nc.tensor.matmul(dp[:, :], ones_t[:mn, 0:1],
                                     gshA[m0][:, g*2*L:(g+1)*2*L],
                                     start=(i == 0), stop=(i == 1))
                nc.vector.tensor_copy(d_sb[:, g*2*L:(g+1)*2*L], dp[:, :])

            part = sb.tile([1, L], dt, tag="part")
            nc.vector.tensor_add(part[:, :], d_sb[:, 0:L], d_sb[:, L:2*L])
            part2 = sb.tile([1, L], dt, tag="part2")
            nc.vector.tensor_add(part2[:, :], d_sb[:, 2*L:3*L], d_sb[:, 3*L:4*L])
            nc.vector.tensor_add(part[:, :], part[:, :], part2[:, :])
            arin_inst = nc.gpsimd.dma_start(out=arin[:, :], in_=part[:, :])
            nc.gpsimd.collective_compute(
                "AllReduce", Alu.add,
                replica_groups=[list(range(N_CORES))],
                ins=[arin[:, :]], outs=[arout[:, :]])

            # ---------- during-CC: d4 scatter + exp ----------
            d4 = sb.tile([PER, L], dt, tag="d4")
            nc.gpsimd.dma_start(
                out=d4[:, :],
                in_=d_sb[:, :].rearrange("p (b l) -> p b l", b=PER))
            e4 = sb.tile([PER, L], dt, tag="e4")
            nc.scalar.activation(e4[:, :], d4[:, :], Exp, bias=0.0, scale=1.0 / HE)

            # ---------- W-phase loads (delayed behind CC input) ----------
            kf_t = sb.tile([128, PER * 4 * L], dtb, tag="kfA", name="kfA")
            vt_t = sb.tile([128, PER * 4 * L], dtb, tag="vtA", name="vtA")
            qr_t = sb.tile([128, PER * 2 * HE], dtb, tag="qrA", name="qrA")
            i1 = nc.sync.dma_start(out=kf_t[:, :], in_=kf[:, :])
            i2 = nc.scalar.dma_start(out=vt_t[:, :], in_=vt[:, :])
            i3 = nc.scalar.dma_start(out=qr_t[:, :], in_=qr[:, :])
            for ii in (i1, i2, i3):
                add_dep_helper(ii.ins, arin_inst.ins, sync=True,
                               reason="delay W loads past CC input")

            dummies(14)

            # ---------- W_b (inside CC flight) ----------
            for b in range(PER):
                for qi, (m0, mn) in enumerate(PCH):
                    wp = ps.tile([mn, L], dt, tag="mm", bufs=3)
                    for i in range(4):
                        nc.tensor.matmul(
                            wp[:, :],
                            slc(kf_t, b, i*L + m0, i*L + m0 + mn),
                            slc(vt_t, b, i*L, (i+1)*L),
                            start=(i == 0), stop=(i == 3))
                    ws = sb.tile([mn, 2 * L], dtb, tag=f"ws{m0}", bufs=2,
                                 name=f"ws{b}_{m0}")
                    nc.vector.tensor_copy(
                        ws[:, :].rearrange("p (r l) -> p r l", r=2),
                        wp[:, :].unsqueeze(1).broadcast_to((mn, 2, L)))
                    eng = nc.sync if qi == 0 else nc.scalar
                    eng.dma_start(
                        out=AP(tensor=wsc, offset=b * BSTR + m0 * 384,
                               ap=[[384, mn], [1, 2 * L]]),
                        in_=ws[:, :])
            wshA = {}
            for qi, (m0, mn) in enumerate(PCH):
                t = sb.tile([mn, PER * L], dtb, tag=f"wshA{m0}")
                eng = nc.sync if qi == 0 else nc.scalar
                eng.dma_start(
                    out=t[:, :].rearrange("p (b l) -> p b l", b=PER),
                    in_=AP(tensor=wsc, offset=m0 * 385,
                           ap=[[385, mn], [BSTR, PER], [1, L]]))
                wshA[m0] = t

            # keep PE hot through the remaining CC flight
            dummies(130)

            # ---------- post-CC: mask + normalize -> c (PER x L) ----------
            gsum4 = sb.tile([PER, L], dt, tag="gsum4")
            nc.gpsimd.dma_start(out=gsum4[:, :],
                                in_=AP(tensor=arout, offset=0,
                                       ap=[[0, PER], [1, L]]))
            mx = sb.tile([PER, 8], dt, tag="mx")
            nc.vector.max(out=mx[:, :], in_=gsum4[:, :])
            me = sb.tile([PER, L], dt, tag="me")
            nc.vector.tensor_scalar(out=me[:, :], in0=gsum4[:, :],
                                    scalar1=mx[:, 4:5], scalar2=None,
                                    op0=Alu.is_ge)
            nc.vector.tensor_tensor(out=me[:, :], in0=me[:, :], in1=e4[:, :],
                                    op=Alu.mult)
            z = sb.tile([PER, 1], dt, tag="z")
            nc.vector.tensor_reduce(out=z[:, :], in_=me[:, :], axis=Ax.X,
                                    op=Alu.add)
            zr = sb.tile([PER, 1], dt, tag="zr")
            nc.vector.reciprocal(zr[:, :], z[:, :])
            c4 = sb.tile([PER, L], dt, tag="c4")
            nc.vector.tensor_scalar(out=c4[:, :], in0=me[:, :],
                                    scalar1=zr[:, :], scalar2=None,
                                    op0=Alu.mult)
            c4t = sb.tile([PER, 3 * L], dtb, tag="c4t")
            nc.vector.tensor_copy(
                c4t[:, :].rearrange("p (r l) -> p r l", r=3),
                c4[:, :].unsqueeze(1).broadcast_to((PER, 3, L)))
            nc.sync.dma_start(
                out=AP(tensor=csc, offset=0, ap=[[576, PER], [1, 3 * L]]),
                in_=c4t[:, :])
            h1 = {}
            for qi, (m0, mn) in enumerate(PCH):
                t = sb.tile([mn, PER * L], dtb, tag=f"h1_{m0}", name=f"h1_{m0}")
                eng = nc.sync if qi == 0 else nc.scalar
                eng.dma_start(
                    out=t[:, :].rearrange("p (b l) -> p b l", b=PER),
                    in_=AP(tensor=csc, offset=1 + m0,
                           ap=[[1, mn], [576, PER], [1, L]]))
                h1[m0] = t

            # ---------- T1 for all batches first (roundtrips pipeline) ----
            for b in range(PER):
                for qi, (m0, mn) in enumerate(PCH):
                    tp = ps.tile([mn, L], dt, tag="mm", bufs=3)
                    for i, (u0, un) in enumerate(PCH):
                        nc.tensor.matmul(tp[:, :],
                                         h1[u0][:, b*L + m0 : b*L + m0 + mn],
                                         wshA[u0][:, b*L:(b+1)*L],
                                         start=(i == 0), stop=(i == 1))
                    ts_ = sb.tile([mn, 2 * L], dtb, tag=f"ts{m0}", bufs=2,
                                  name=f"ts{b}_{m0}")
                    nc.vector.tensor_copy(
                        ts_[:, :].rearrange("p (r l) -> p r l", r=2),
                        tp[:, :].unsqueeze(1).broadcast_to((mn, 2, L)))
                    eng = nc.sync if qi == 0 else nc.scalar
                    eng.dma_start(
                        out=AP(tensor=msc, offset=b * BSTR + m0 * 384,
                               ap=[[384, mn], [1, 2 * L]]),
                        in_=ts_[:, :])
            # merged mrev read per batch (one trigger; k=1 upper partitions
            # read harmless garbage that no matmul consumes)
            mrev = {}
            for b in range(PER):
                t = sb.tile([128, 2 * L], dtb, tag="mrev", bufs=4,
                            name=f"mrev{b}")
                nc.scalar.dma_start(
                    out=t[:, :].rearrange("p (k l) -> p k l", k=2),
                    in_=AP(tensor=msc, offset=b * BSTR + 1,
                           ap=[[385, 128], [128 * 385, 2], [1, L]]))
                mrev[b] = t

            # ---------- final per b ----------
            for b in range(PER):
                for qi, (l0, ln) in enumerate(PCH):
                    op_ = ps.tile([ln, HE], dt, tag="op", bufs=2)
                    for i, (i0, in_n) in enumerate(PCH):
                        nc.tensor.matmul(
                            op_[:, :],
                            mrev[b][0:in_n, i*L + l0 : i*L + l0 + ln],
                            qr_t[0:in_n, b*2*HE + i*HE : b*2*HE + (i+1)*HE],
                            start=(i == 0), stop=(i == 1))
                    os_ = sb.tile([ln, HE], dtb, tag=f"os{l0}", bufs=2,
                                  name=f"os{b}_{l0}")
                    nc.vector.tensor_copy(os_[:, :], op_[:, :])
                    eng = nc.sync if qi == 0 else nc.scalar
                    eng.dma_start(out=out[b, l0:l0+ln, :], in_=os_[:, :])

    nc.finalize()
    return nc


def _get_nc():
    if "nc" not in _compiled:
        _compiled["nc"] = _build()
    return _compiled["nc"]


def _pack_chunks(mat):
    # (HE, L) -> (128, 4*L): column block i holds channels [i*128,(i+1)*128)
    return np.ascontiguousarray(
        mat.reshape(4, 128, L).transpose(1, 0, 2).reshape(128, 4 * L))


def kernel(queries, keys, values, adj, attn_mask):
    from concourse.bass_utils import run_bass_kernel_spmd

    queries = np.ascontiguousarray(np.asarray(queries, dtype=np.float32))
    keys = np.ascontiguousarray(np.asarray(keys, dtype=np.float32))
    values = np.ascontiguousarray(np.asarray(values, dtype=np.float32))

    nc = _get_nc()
    in_maps = []
    for c in range(N_CORES):
        sl = slice(c * PER, (c + 1) * PER)
        q = queries[sl].reshape(PER, L, HE)
        k = keys[sl].reshape(PER, L, HE)
        v = values[sl]
        kt = np.empty((128, PER * 4 * L), BF)
        qt = np.empty((128, PER * 4 * L), BF)
        kfp = np.empty((128, PER * 4 * L), BF)
        vtp = np.empty((128, PER * 4 * L), BF)
        qrp = np.zeros((128, PER * 2 * HE), BF)
        for b in range(PER):
            cb = slice(b * 4 * L, (b + 1) * 4 * L)
            kt[:, cb] = _pack_chunks(k[b].T.astype(BF))
            qt[:, cb] = _pack_chunks(q[b].T.astype(BF))
            kfp[:, cb] = _pack_chunks(k[b].reshape(HE, L).astype(BF))
            vtp[:, cb] = _pack_chunks(v[b].transpose(1, 2, 0)
                                      .reshape(HE, L).astype(BF))
            qsp = q[b].reshape(HE, L)
            qr_mat = qsp.T[::-1, :].astype(BF)
            qrp[:, b*2*HE : b*2*HE + HE] = qr_mat[0:128, :]
            qrp[0:64, b*2*HE + HE : (b+1)*2*HE] = qr_mat[128:192, :]
        in_maps.append({
            "kt": kt, "qt": qt, "kf": kfp, "vt": vtp, "qr": qrp,
            "ones_in": np.ones((128, 512), BF),
        })

    res = run_bass_kernel_spmd(nc, in_maps, list(range(N_CORES)),
                               **_compiled.get("run_kwargs", {}))
    _compiled["last_result"] = res
    outs = [np.asarray(res.results[c]["out"], dtype=np.float32)
            .reshape(PER, L, H, E) for c in range(N_CORES)]
    return np.concatenate(outs, axis=0)
